# revision 15
# baseline (speedup 1.0000x reference)
"""Trainium2 Bass kernel for nn_DifferentiableReconstruction.

recon[b,v] = sum_t w[b,t,v]*im[b,t] / sum_t w[b,t,v]
  w = exp(1/(dist+eps)),  dist = ||grid[v] - c[b,t]||,  c = gathered transform xyz
  im[b,t] = mean over (C,H,W) of slices[b, idx[b,t]]

Single fused SPMD launch on 8 NeuronCores:
  - slice means: B*T=256 slices sharded 32/core; per-core partial sums,
    AllGather of the 32 per-core block sums, then an on-device one-hot
    permutation matmul (indices baked host-side) produces im[b,:].
  - reconstruction: voxel dim V=64^3 sharded 32768/core (contiguous x-slabs).
    dist2 via K=14 bf16 matmul (expansion g2+c2-2g.c, hi/lo bf16 splits ->
    ~fp32-exact), d=Sqrt(dist2) on ACT, u=1/d via DVE bit-trick reciprocal,
    w=exp(u) on ACT (fp16), T-reduction as w-as-lhsT matmuls against
    [im_hi, im_lo, 1], divide on DVE, PE transpose for contiguous output.
"""

import os
import sys
import types

for _p in ("/opt/trn_rl_repo", "/root/.axon_site", "/root/.axon_site/_ro/pypackages"):
    if _p not in sys.path and os.path.isdir(_p):
        sys.path.append(_p)

import numpy as np

import concourse.bacc as bacc
import concourse.bass as bass
import concourse.tile as tile
import concourse.mybir as mybir
from concourse.bass_utils import run_bass_kernel_spmd

VOLX = 64
V = VOLX * VOLX * VOLX            # 262144
B, T, C, H, W = 2, 128, 1, 256, 256
HWN = C * H * W                   # 65536
N_CORES = 8
VLOC = V // N_CORES               # 32768
CENTER = (VOLX - 1) / 2.0         # 31.5
KD = 14
F32 = mybir.dt.float32
BF16 = mybir.dt.bfloat16
FP16 = mybir.dt.float16
AF = mybir.ActivationFunctionType

LAST_INFO = {}


def _install_trace_shim():
    if "antenv.axon_hooks" in sys.modules:
        return
    try:
        from trn_agent_boot.trn_boot import _ntff_profile_via_ctypes
        hook = _ntff_profile_via_ctypes("/opt/axon/libaxon_pjrt.so")
    except Exception:
        return
    mod = types.ModuleType("antenv.axon_hooks")
    mod._hook = hook
    mod.get_axon_ntff_profile_hook = lambda: mod._hook
    mod.set_axon_ntff_profile_hook = lambda h: setattr(mod, "_hook", h)
    sys.modules["antenv.axon_hooks"] = mod


def _build_nc():
    nc = bacc.Bacc("TRN2", target_bir_lowering=False, debug=False,
                   num_devices=N_CORES)
    sl = nc.dram_tensor("sl", [128, 16384], F32, kind="ExternalInput")
    gaug = nc.dram_tensor("gaug", [KD, VLOC], BF16, kind="ExternalInput")
    caug = nc.dram_tensor("caug", [B, KD, 128], BF16, kind="ExternalInput")
    pmat = nc.dram_tensor("pmat", [B, 128, 128], F32, kind="ExternalInput")
    bsum = nc.dram_tensor("bsum", [128, 32], F32, kind="ExternalInput")
    iden = nc.dram_tensor("iden", [128, 128], F32, kind="ExternalInput")
    recon = nc.dram_tensor("recon", [B, VLOC], F32, kind="ExternalOutput")

    from concourse.dve_ops import (RECIP_APPROX_FAST_CONSTS,
                                   RECIPROCAL_APPROX_FAST)
    _rc = RECIP_APPROX_FAST_CONSTS

    with tile.TileContext(nc) as tc:
        with tc.tile_pool(name="const", bufs=1) as constp, \
             tc.tile_pool(name="slp", bufs=1) as slp, \
             tc.tile_pool(name="gch", bufs=2) as gchp, \
             tc.tile_pool(name="ubuf", bufs=1) as ubufp, \
             tc.tile_pool(name="wt", bufs=2) as wtp, \
             tc.tile_pool(name="d2ps", bufs=2, space="PSUM") as d2psp, \
             tc.tile_pool(name="ndps", bufs=1, space="PSUM") as ndpsp, \
             tc.tile_pool(name="tps", bufs=2, space="PSUM") as tpsp, \
             tc.tile_pool(name="res", bufs=2) as resp, \
             tc.tile_pool(name="ob", bufs=4) as obp, \
             tc.tile_pool(name="dram", bufs=1, space="DRAM") as dramp:

            # ---------------- constants
            cau = constp.tile([KD, B * 128], BF16)
            for b in range(B):
                nc.sync.dma_start(cau[:, b * 128:(b + 1) * 128], caug[b])
            idn = constp.tile([128, 128], F32)
            nc.sync.dma_start(idn[:], iden[:])
            bsm = constp.tile([128, 32], F32)
            nc.sync.dma_start(bsm[:], bsum[:])
            pmt = constp.tile([128, B * 128], F32)
            for b in range(B):
                nc.sync.dma_start(pmt[:, b * 128:(b + 1) * 128], pmat[b])

            # ---------------- reconstruction phase A (both b):
            # dist2 (PE) -> d=sqrt (ACT, one table set) -> u=1/d (DVE bit trick)
            ubuf = ubufp.tile([128, B * VLOC], FP16)
            sqrt_insts = []
            recip_insts = []
            gch_dmas = []
            accs = []
            for b in range(B):
                for gj in range(8):
                    gch = gchp.tile([KD, 4096], BF16)
                    gd = nc.sync.dma_start(
                        gch[:], gaug[:, gj * 4096:(gj + 1) * 4096])
                    gch_dmas.append(gd)
                    if b == 0 and gj == 3 and not accs:
                        # slice partial sums via accumulating SWDGE DMAs,
                        # delayed so gaug wins the DMA bandwidth race at t=0.
                        for c in range(2):
                            at = slp.tile([128, 2048], F32, tag=f"acc{c}")
                            for j in range(4):
                                ch = 2 * j + c
                                di = nc.gpsimd.dma_start(
                                    at[:], sl[:, 2048 * ch:2048 * (ch + 1)],
                                    accum_op=(mybir.AluOpType.bypass if j == 0
                                              else mybir.AluOpType.add))
                                if j == 0:
                                    tile.add_dep_helper(
                                        di.ins, gch_dmas[2].ins,
                                        reason="gaug first on DMA")
                            accs.append(at)
                    for g2 in range(4):
                        ps = d2psp.tile([128, 1024], F32)
                        for h in range(2):
                            cc = g2 * 1024 + h * 512
                            nc.tensor.matmul(
                                ps[:, h * 512:(h + 1) * 512],
                                cau[:, b * 128:(b + 1) * 128],
                                gch[:, cc:cc + 512],
                                start=True, stop=True)
                        base = b * VLOC + gj * 4096 + g2 * 1024
                        dt_ = resp.tile([128, 1024], F32, tag="dt")
                        si = nc.scalar.activation(dt_[:], ps[:], AF.Sqrt)
                        sqrt_insts.append(si)
                        ri = nc.vector._custom_dve(
                            RECIPROCAL_APPROX_FAST,
                            out=ubuf[:, base:base + 1024], in0=dt_[:],
                            s0=_rc["s0"], s1=_rc["s1"], imm2=_rc["imm2"])
                        recip_insts.append(ri)

            # ---------------- means tail: block-sum, AllGather, im gather
            # (emitted after phase A so the PE queue is not head-blocked)
            s128 = constp.tile([128, 1], F32)
            acc2 = constp.tile([128, 2], F32)
            for c in range(2):
                nc.vector.reduce_sum(acc2[:, c:c + 1], accs[c][:],
                                     axis=mybir.AxisListType.X)
            nc.vector.reduce_sum(s128[:], acc2[:], axis=mybir.AxisListType.X)
            p32 = tpsp.tile([32, 1], F32, tag="tp")
            nc.tensor.matmul(p32[:], bsm[:], s128[:], start=True, stop=True)
            p32s = constp.tile([32, 1], F32)
            nc.scalar.copy(p32s[:], p32[:])
            cc_in = dramp.tile([32, 1], F32)
            cc_out = dramp.tile([256, 1], F32)
            nc.sync.dma_start(cc_in[:], p32s[:])
            nc.gpsimd.collective_compute(
                "AllGather", mybir.AluOpType.bypass,
                replica_groups=[list(range(N_CORES))],
                ins=[cc_in.opt()], outs=[cc_out.opt()])
            m_sb = constp.tile([128, 2], F32)
            for b in range(B):
                nc.sync.dma_start(
                    m_sb[:, b:b + 1],
                    cc_out[128 * b:128 * (b + 1)])
            rlh = constp.tile([128, B * 3], FP16)
            im32 = constp.tile([128, B], F32)
            hi32 = constp.tile([128, B], F32)
            lo32 = constp.tile([128, B], F32)
            for b in range(B):
                imp = tpsp.tile([128, 1], F32, tag="tp")
                nc.tensor.matmul(imp[:], pmt[:, b * 128:(b + 1) * 128],
                                 m_sb[:, b:b + 1], start=True, stop=True)
                nc.scalar.copy(im32[:, b:b + 1], imp[:])
                # rlh cols per b: [im_hi fp16, im_lo fp16, ones]
                nc.scalar.copy(rlh[:, 3 * b:3 * b + 1], im32[:, b:b + 1])
                nc.scalar.copy(hi32[:, b:b + 1], rlh[:, 3 * b:3 * b + 1])
                with nc.allow_low_precision(reason="fp16 lo-part split"):
                    nc.vector.tensor_sub(
                        lo32[:, b:b + 1],
                        im32[:, b:b + 1], hi32[:, b:b + 1])
                nc.scalar.copy(rlh[:, 3 * b + 1:3 * b + 2], lo32[:, b:b + 1])
                nc.gpsimd.memset(rlh[:, 3 * b + 2:3 * b + 3], 1.0)

            # ---------------- phase B (both b): w = exp(u) + T-reduction
            prev = sqrt_insts[-1].ins
            for b in range(B):
                nd = ndpsp.tile([128, 1024], F32, tag="nd")
                for q in range(VLOC // 4096):
                    wt = wtp.tile([128, 4096], FP16, tag="wt")
                    ei = nc.scalar.activation(
                        wt[:], ubuf[:, b * VLOC + q * 4096:
                                    b * VLOC + (q + 1) * 4096], AF.Exp)
                    tile.add_dep_helper(ei.ins, prev,
                                        reason="act-table order")
                    for s in range(32):
                        sub = q * 32 + s
                        nc.tensor.matmul(
                            nd[:, 4 * sub:4 * sub + 3],
                            wt[:, 128 * s:128 * (s + 1)],
                            rlh[:, 3 * b:3 * (b + 1)],
                            start=True, stop=True)

                # phase C: recon = (num_hi + num_lo) / den
                nd_v = nd[:].rearrange("p (n four) -> p n four", four=4)
                denr = resp.tile([128, 256], F32, tag="denr")
                nc.vector.reciprocal(denr[:], nd_v[:, :, 2])
                r0 = resp.tile([128, 256], F32, tag="r0")
                nc.vector.tensor_mul(r0[:], nd_v[:, :, 0], denr[:])
                r1 = resp.tile([128, 256], F32, tag="r1")
                nc.vector.tensor_mul(r1[:], nd_v[:, :, 1], denr[:])
                res = resp.tile([128, 256], F32, tag="res")
                nc.vector.tensor_add(res[:], r0[:], r1[:])

                # phase D: PE transpose -> contiguous DMA out
                for h in range(2):
                    tp = tpsp.tile([128, 128], F32, tag="tp")
                    nc.tensor.transpose(tp[:], res[:, h * 128:(h + 1) * 128],
                                        idn[:])
                    ob = obp.tile([128, 128], F32)
                    nc.vector.tensor_copy(ob[:], tp[:])
                    dv = recon[b, h * 16384:(h + 1) * 16384]
                    dv = dv.rearrange("(s p) -> s p", p=128)
                    nc.sync.dma_start(dv, ob[:])
    nc.compile()
    return nc


_NC_CACHE = {}


def _split3_bf16(x):
    import ml_dtypes
    a = x.astype(ml_dtypes.bfloat16)
    r1 = x - a.astype(np.float64)
    b = r1.astype(ml_dtypes.bfloat16)
    r2 = r1 - b.astype(np.float64)
    c = r2.astype(ml_dtypes.bfloat16)
    return a, b, c


def kernel(slices, transforms, slice_indices):
    _install_trace_shim()
    import ml_dtypes

    trace = bool(os.environ.get("BASS_TRACE"))
    slices = np.ascontiguousarray(slices, dtype=np.float32)
    transforms = np.asarray(transforms, dtype=np.float32)
    idx = np.asarray(slice_indices).astype(np.int64)

    if "nc" not in _NC_CACHE:
        _NC_CACHE["nc"] = _build_nc()
    nc = _NC_CACHE["nc"]

    # ---- host prep (sharding + tiny per-(b,t) coefficient builds)
    flat = slices.reshape(B * T, HWN)

    sel_t = np.take_along_axis(transforms, idx[:, :, None], axis=1)[..., :3]
    cxyz = sel_t.astype(np.float64) - CENTER
    c2 = (cxyz ** 2).sum(-1)
    caug = np.zeros((B, KD, 128), dtype=np.float64)
    for ax in range(3):
        p1, p2, p3 = _split3_bf16(-2.0 * cxyz[:, :, ax])
        caug[:, 3 * ax + 0] = p1.astype(np.float64)
        caug[:, 3 * ax + 1] = p2.astype(np.float64)
        caug[:, 3 * ax + 2] = p3.astype(np.float64)
    caug[:, 9] = 1.0
    caug[:, 10] = 1.0
    q1, q2, q3 = _split3_bf16(c2)
    caug[:, 11] = q1.astype(np.float64)
    caug[:, 12] = q2.astype(np.float64)
    caug[:, 13] = q3.astype(np.float64)
    caug_bf = caug.astype(ml_dtypes.bfloat16)

    # one-hot permutation (gather) matrices: im[b,t] = sum_j pmat[b,j,t]*m[b,j]
    pm = np.zeros((B, 128, 128), dtype=np.float32)
    for b in range(B):
        pm[b, idx[b, :], np.arange(T)] = 1.0 / HWN
    bs = np.zeros((128, 32), dtype=np.float32)
    bs[np.arange(128), np.arange(128) // 4] = 1.0
    iden = np.eye(128, dtype=np.float32)

    yz = np.arange(4096)
    gy = (yz // 64).astype(np.float64) - CENTER
    gz = (yz % 64).astype(np.float64) - CENTER
    gaug_list = []
    for k in range(N_CORES):
        ga = np.zeros((KD, VLOC), dtype=np.float64)
        for xi in range(8):
            x = 8 * k + xi
            gx = np.full(4096, x - CENTER)
            g2 = gx * gx + gy * gy + gz * gz
            g2h = g2.astype(ml_dtypes.bfloat16).astype(np.float64)
            g2l = g2 - g2h
            sl_ = slice(4096 * xi, 4096 * (xi + 1))
            for r in range(3):
                ga[0 + r, sl_] = gx
                ga[3 + r, sl_] = gy
                ga[6 + r, sl_] = gz
            ga[9, sl_] = g2h
            ga[10, sl_] = g2l
            ga[11:14, sl_] = 1.0
        gaug_list.append(ga.astype(ml_dtypes.bfloat16))

    in_maps = []
    for k in range(N_CORES):
        in_maps.append({
            "sl": np.ascontiguousarray(
                flat[32 * k:32 * (k + 1)].reshape(128, 16384)),
            "gaug": gaug_list[k],
            "caug": caug_bf,
            "pmat": pm,
            "bsum": bs,
            "iden": iden,
        })

    r = run_bass_kernel_spmd(nc, in_maps, core_ids=list(range(N_CORES)),
                             trace=trace)

    out = np.empty((B, VOLX, VOLX, VOLX), dtype=np.float32)
    for k in range(N_CORES):
        rk = r.results[k]["recon"]
        out[:, 8 * k:8 * (k + 1)] = rk.reshape(B, 8, VOLX, VOLX)

    LAST_INFO["r2"] = r
    LAST_INFO["means_ns"] = 0
    LAST_INFO["recon_ns"] = r.exec_time_ns
    LAST_INFO["total_ns"] = r.exec_time_ns
    return out.reshape(B, 1, VOLX, VOLX, VOLX)


# revision 16
# speedup vs baseline: 1.1747x; 1.1747x over previous
"""Trainium2 Bass kernel for nn_DifferentiableReconstruction.

recon[b,v] = sum_t w[b,t,v]*im[b,t] / sum_t w[b,t,v]
  w = exp(1/(dist+eps)),  dist = ||grid[v] - c[b,t]||,  c = gathered transform xyz
  im[b,t] = mean over (C,H,W) of slices[b, idx[b,t]]

Single fused SPMD launch on 8 NeuronCores:
  - slice means: B*T=256 slices sharded 32/core; per-core partial sums,
    AllGather of the 32 per-core block sums, then an on-device one-hot
    permutation matmul (indices baked host-side) produces im[b,:].
  - reconstruction: voxel dim V=64^3 sharded 32768/core (contiguous x-slabs).
    dist2 via K=14 bf16 matmul (expansion g2+c2-2g.c, hi/lo bf16 splits ->
    ~fp32-exact), d=Sqrt(dist2) on ACT, u=1/d via DVE bit-trick reciprocal,
    w=exp(u) on ACT (fp16), T-reduction as w-as-lhsT matmuls against
    [im_hi, im_lo, 1], divide on DVE, PE transpose for contiguous output.
"""

import os
import sys
import types

for _p in ("/opt/trn_rl_repo", "/root/.axon_site", "/root/.axon_site/_ro/pypackages"):
    if _p not in sys.path and os.path.isdir(_p):
        sys.path.append(_p)

import numpy as np

import concourse.bacc as bacc
import concourse.bass as bass
import concourse.tile as tile
import concourse.mybir as mybir
from concourse.bass_utils import run_bass_kernel_spmd

VOLX = 64
V = VOLX * VOLX * VOLX            # 262144
B, T, C, H, W = 2, 128, 1, 256, 256
HWN = C * H * W                   # 65536
N_CORES = 8
VLOC = V // N_CORES               # 32768
CENTER = (VOLX - 1) / 2.0         # 31.5
KD = 14
F32 = mybir.dt.float32
BF16 = mybir.dt.bfloat16
FP16 = mybir.dt.float16
AF = mybir.ActivationFunctionType

LAST_INFO = {}


def _install_trace_shim():
    if "antenv.axon_hooks" in sys.modules:
        return
    try:
        from trn_agent_boot.trn_boot import _ntff_profile_via_ctypes
        hook = _ntff_profile_via_ctypes("/opt/axon/libaxon_pjrt.so")
    except Exception:
        return
    mod = types.ModuleType("antenv.axon_hooks")
    mod._hook = hook
    mod.get_axon_ntff_profile_hook = lambda: mod._hook
    mod.set_axon_ntff_profile_hook = lambda h: setattr(mod, "_hook", h)
    sys.modules["antenv.axon_hooks"] = mod


def _build_nc():
    nc = bacc.Bacc("TRN2", target_bir_lowering=False, debug=False,
                   num_devices=N_CORES)
    sl = nc.dram_tensor("sl", [128, 16384], F32, kind="ExternalInput")
    gaug = nc.dram_tensor("gaug", [KD, VLOC], BF16, kind="ExternalInput")
    caug = nc.dram_tensor("caug", [B, KD, 128], BF16, kind="ExternalInput")
    pmat = nc.dram_tensor("pmat", [B, 128, 128], F32, kind="ExternalInput")
    bsum = nc.dram_tensor("bsum", [128, 32], F32, kind="ExternalInput")
    iden = nc.dram_tensor("iden", [128, 128], F32, kind="ExternalInput")
    recon = nc.dram_tensor("recon", [B, VLOC], F32, kind="ExternalOutput")

    from concourse.dve_ops import (RECIP_APPROX_FAST_CONSTS,
                                   RECIPROCAL_APPROX_FAST)
    _rc = RECIP_APPROX_FAST_CONSTS

    with tile.TileContext(nc) as tc:
        with tc.tile_pool(name="const", bufs=1) as constp, \
             tc.tile_pool(name="slp", bufs=1) as slp, \
             tc.tile_pool(name="gch", bufs=2) as gchp, \
             tc.tile_pool(name="ubuf", bufs=1) as ubufp, \
             tc.tile_pool(name="wt", bufs=2) as wtp, \
             tc.tile_pool(name="d2ps", bufs=3, space="PSUM") as d2psp, \
             tc.tile_pool(name="ndps", bufs=1, space="PSUM") as ndpsp, \
             tc.tile_pool(name="tps", bufs=1, space="PSUM") as tpsp, \
             tc.tile_pool(name="res", bufs=2) as resp, \
             tc.tile_pool(name="ob", bufs=4) as obp, \
             tc.tile_pool(name="dram", bufs=1, space="DRAM") as dramp:

            # ---------------- constants
            cau = constp.tile([KD, B * 128], BF16)
            for b in range(B):
                nc.sync.dma_start(cau[:, b * 128:(b + 1) * 128], caug[b])
            idn = constp.tile([128, 128], F32)
            nc.sync.dma_start(idn[:], iden[:])
            bsm = constp.tile([128, 32], F32)
            nc.sync.dma_start(bsm[:], bsum[:])
            pmt = constp.tile([128, B * 128], F32)
            for b in range(B):
                nc.sync.dma_start(pmt[:, b * 128:(b + 1) * 128], pmat[b])

            # ---------------- reconstruction phase A (both b):
            # dist2 (PE) -> d=sqrt (ACT, one table set) -> u=1/d (DVE bit trick)
            ubuf = ubufp.tile([128, B * VLOC], FP16)
            sqrt_insts = []
            recip_insts = []
            gch_dmas = []
            accs = []
            for b in range(B):
                for gj in range(8):
                    gch = gchp.tile([KD, 4096], BF16)
                    gd = nc.sync.dma_start(
                        gch[:], gaug[:, gj * 4096:(gj + 1) * 4096])
                    gch_dmas.append(gd)
                    if b == 0 and gj == 3 and not accs:
                        # slice partial sums via accumulating SWDGE DMAs,
                        # delayed so gaug wins the DMA bandwidth race at t=0.
                        for c in range(2):
                            at = slp.tile([128, 2048], F32, tag=f"acc{c}")
                            for j in range(4):
                                ch = 2 * j + c
                                di = nc.gpsimd.dma_start(
                                    at[:], sl[:, 2048 * ch:2048 * (ch + 1)],
                                    accum_op=(mybir.AluOpType.bypass if j == 0
                                              else mybir.AluOpType.add))
                                if j == 0:
                                    tile.add_dep_helper(
                                        di.ins, gch_dmas[2].ins,
                                        reason="gaug first on DMA")
                            accs.append(at)
                    for g2 in range(4):
                        ps = d2psp.tile([128, 1024], F32)
                        for h in range(2):
                            cc = g2 * 1024 + h * 512
                            nc.tensor.matmul(
                                ps[:, h * 512:(h + 1) * 512],
                                cau[:, b * 128:(b + 1) * 128],
                                gch[:, cc:cc + 512],
                                start=True, stop=True)
                        base = b * VLOC + gj * 4096 + g2 * 1024
                        dt_ = resp.tile([128, 1024], F32, tag="dt")
                        si = nc.scalar.activation(dt_[:], ps[:], AF.Sqrt)
                        sqrt_insts.append(si)
                        ri = nc.vector._custom_dve(
                            RECIPROCAL_APPROX_FAST,
                            out=ubuf[:, base:base + 1024], in0=dt_[:],
                            s0=_rc["s0"], s1=_rc["s1"], imm2=_rc["imm2"])
                        recip_insts.append(ri)

            # ---------------- means tail: block-sum, AllGather, im gather
            # (emitted after phase A so the PE queue is not head-blocked)
            s128 = constp.tile([128, 1], F32)
            acc2 = constp.tile([128, 2], F32)
            for c in range(2):
                nc.vector.reduce_sum(acc2[:, c:c + 1], accs[c][:],
                                     axis=mybir.AxisListType.X)
            nc.vector.reduce_sum(s128[:], acc2[:], axis=mybir.AxisListType.X)
            p32 = tpsp.tile([32, 1], F32, tag="tp")
            nc.tensor.matmul(p32[:], bsm[:], s128[:], start=True, stop=True)
            p32s = constp.tile([32, 1], F32)
            nc.scalar.copy(p32s[:], p32[:])
            cc_in = dramp.tile([32, 1], F32)
            cc_out = dramp.tile([256, 1], F32)
            nc.sync.dma_start(cc_in[:], p32s[:])
            nc.gpsimd.collective_compute(
                "AllGather", mybir.AluOpType.bypass,
                replica_groups=[list(range(N_CORES))],
                ins=[cc_in.opt()], outs=[cc_out.opt()])
            m_sb = constp.tile([128, 2], F32)
            for b in range(B):
                nc.sync.dma_start(
                    m_sb[:, b:b + 1],
                    cc_out[128 * b:128 * (b + 1)])
            rlh = constp.tile([128, B * 3], FP16)
            im32 = constp.tile([128, B], F32)
            hi32 = constp.tile([128, B], F32)
            lo32 = constp.tile([128, B], F32)
            for b in range(B):
                imp = tpsp.tile([128, 1], F32, tag="tp")
                nc.tensor.matmul(imp[:], pmt[:, b * 128:(b + 1) * 128],
                                 m_sb[:, b:b + 1], start=True, stop=True)
                nc.scalar.copy(im32[:, b:b + 1], imp[:])
                # rlh cols per b: [im_hi fp16, im_lo fp16, ones]
                nc.scalar.copy(rlh[:, 3 * b:3 * b + 1], im32[:, b:b + 1])
                nc.scalar.copy(hi32[:, b:b + 1], rlh[:, 3 * b:3 * b + 1])
                with nc.allow_low_precision(reason="fp16 lo-part split"):
                    nc.vector.tensor_sub(
                        lo32[:, b:b + 1],
                        im32[:, b:b + 1], hi32[:, b:b + 1])
                nc.scalar.copy(rlh[:, 3 * b + 1:3 * b + 2], lo32[:, b:b + 1])
                nc.gpsimd.memset(rlh[:, 3 * b + 2:3 * b + 3], 1.0)

            # ---------------- phase B (both b): w = exp(u) + T-reduction
            prev = sqrt_insts[-1].ins
            for b in range(B):
                for half in range(2):
                    nd = ndpsp.tile([128, 512], F32, tag="nd")
                    for q2 in range(4):
                        q = half * 4 + q2
                        wt = wtp.tile([128, 4096], FP16, tag="wt")
                        ei = nc.scalar.activation(
                            wt[:], ubuf[:, b * VLOC + q * 4096:
                                        b * VLOC + (q + 1) * 4096], AF.Exp)
                        tile.add_dep_helper(ei.ins, prev,
                                            reason="act-table order")
                        for s in range(32):
                            sub = q2 * 32 + s
                            nc.tensor.matmul(
                                nd[:, 4 * sub:4 * sub + 3],
                                wt[:, 128 * s:128 * (s + 1)],
                                rlh[:, 3 * b:3 * (b + 1)],
                                start=True, stop=True)

                    # phase C: recon = (num_hi + num_lo) / den
                    nd_v = nd[:].rearrange("p (n four) -> p n four", four=4)
                    denr = resp.tile([128, 128], F32, tag="denr")
                    nc.vector.reciprocal(denr[:], nd_v[:, :, 2])
                    r0 = resp.tile([128, 128], F32, tag="r0")
                    nc.vector.tensor_mul(r0[:], nd_v[:, :, 0], denr[:])
                    r1 = resp.tile([128, 128], F32, tag="r1")
                    nc.vector.tensor_mul(r1[:], nd_v[:, :, 1], denr[:])
                    res = resp.tile([128, 128], F32, tag="res")
                    nc.vector.tensor_add(res[:], r0[:], r1[:])

                    # phase D: PE transpose -> contiguous DMA out
                    tp = tpsp.tile([128, 128], F32, tag="tp")
                    nc.tensor.transpose(tp[:], res[:], idn[:])
                    ob = obp.tile([128, 128], F32)
                    nc.vector.tensor_copy(ob[:], tp[:])
                    dv = recon[b, half * 16384:(half + 1) * 16384]
                    dv = dv.rearrange("(s p) -> s p", p=128)
                    nc.sync.dma_start(dv, ob[:])
    nc.compile()
    return nc


_NC_CACHE = {}


def _split3_bf16(x):
    import ml_dtypes
    a = x.astype(ml_dtypes.bfloat16)
    r1 = x - a.astype(np.float64)
    b = r1.astype(ml_dtypes.bfloat16)
    r2 = r1 - b.astype(np.float64)
    c = r2.astype(ml_dtypes.bfloat16)
    return a, b, c


def kernel(slices, transforms, slice_indices):
    _install_trace_shim()
    import ml_dtypes

    trace = bool(os.environ.get("BASS_TRACE"))
    slices = np.ascontiguousarray(slices, dtype=np.float32)
    transforms = np.asarray(transforms, dtype=np.float32)
    idx = np.asarray(slice_indices).astype(np.int64)

    if "nc" not in _NC_CACHE:
        _NC_CACHE["nc"] = _build_nc()
    nc = _NC_CACHE["nc"]

    # ---- host prep (sharding + tiny per-(b,t) coefficient builds)
    flat = slices.reshape(B * T, HWN)

    sel_t = np.take_along_axis(transforms, idx[:, :, None], axis=1)[..., :3]
    cxyz = sel_t.astype(np.float64) - CENTER
    c2 = (cxyz ** 2).sum(-1)
    caug = np.zeros((B, KD, 128), dtype=np.float64)
    for ax in range(3):
        p1, p2, p3 = _split3_bf16(-2.0 * cxyz[:, :, ax])
        caug[:, 3 * ax + 0] = p1.astype(np.float64)
        caug[:, 3 * ax + 1] = p2.astype(np.float64)
        caug[:, 3 * ax + 2] = p3.astype(np.float64)
    caug[:, 9] = 1.0
    caug[:, 10] = 1.0
    q1, q2, q3 = _split3_bf16(c2)
    caug[:, 11] = q1.astype(np.float64)
    caug[:, 12] = q2.astype(np.float64)
    caug[:, 13] = q3.astype(np.float64)
    caug_bf = caug.astype(ml_dtypes.bfloat16)

    # one-hot permutation (gather) matrices: im[b,t] = sum_j pmat[b,j,t]*m[b,j]
    pm = np.zeros((B, 128, 128), dtype=np.float32)
    for b in range(B):
        pm[b, idx[b, :], np.arange(T)] = 1.0 / HWN
    bs = np.zeros((128, 32), dtype=np.float32)
    bs[np.arange(128), np.arange(128) // 4] = 1.0
    iden = np.eye(128, dtype=np.float32)

    yz = np.arange(4096)
    gy = (yz // 64).astype(np.float64) - CENTER
    gz = (yz % 64).astype(np.float64) - CENTER
    gaug_list = []
    for k in range(N_CORES):
        ga = np.zeros((KD, VLOC), dtype=np.float64)
        for xi in range(8):
            x = 8 * k + xi
            gx = np.full(4096, x - CENTER)
            g2 = gx * gx + gy * gy + gz * gz
            g2h = g2.astype(ml_dtypes.bfloat16).astype(np.float64)
            g2l = g2 - g2h
            sl_ = slice(4096 * xi, 4096 * (xi + 1))
            for r in range(3):
                ga[0 + r, sl_] = gx
                ga[3 + r, sl_] = gy
                ga[6 + r, sl_] = gz
            ga[9, sl_] = g2h
            ga[10, sl_] = g2l
            ga[11:14, sl_] = 1.0
        gaug_list.append(ga.astype(ml_dtypes.bfloat16))

    in_maps = []
    for k in range(N_CORES):
        in_maps.append({
            "sl": np.ascontiguousarray(
                flat[32 * k:32 * (k + 1)].reshape(128, 16384)),
            "gaug": gaug_list[k],
            "caug": caug_bf,
            "pmat": pm,
            "bsum": bs,
            "iden": iden,
        })

    r = run_bass_kernel_spmd(nc, in_maps, core_ids=list(range(N_CORES)),
                             trace=trace)

    out = np.empty((B, VOLX, VOLX, VOLX), dtype=np.float32)
    for k in range(N_CORES):
        rk = r.results[k]["recon"]
        out[:, 8 * k:8 * (k + 1)] = rk.reshape(B, 8, VOLX, VOLX)

    LAST_INFO["r2"] = r
    LAST_INFO["means_ns"] = 0
    LAST_INFO["recon_ns"] = r.exec_time_ns
    LAST_INFO["total_ns"] = r.exec_time_ns
    return out.reshape(B, 1, VOLX, VOLX, VOLX)
